# revision 1
# baseline (speedup 1.0000x reference)
"""Distributed ADC-GNN kernel for 8 TRN2 NeuronCores.

- Nodes sharded contiguously (NS/core, NP=8*NS padded). Edges partitioned by
  dst shard, dst-sorted, per-128-node-block, split by src table half
  (A = local rows [0,NSA), B = [NSA,NS) of each rank), padded to 128-edge
  chunks shared across cores (SPMD).
- y tables: two Shared DRAM tables per stage (halves all-gathered
  separately so gathers pipeline with the second AllGather).
- SpMM: batched relaxed dma_gather (64-elem bf16 payload @ 128-elem stride,
  4 SWDGE queues round-robin) + batched one-hot S on DVE + PE matmul
  accumulation per block.
- MLPs feature-major with fused ACT bias+relu; attention/fusion per-super
  so the tail overlaps spmm2.
"""
import inspect
import numpy as np
import ml_dtypes

import concourse.bass as bass
import concourse.bacc as bacc
import concourse.mybir as mybir
import concourse.tile as tile

BF16 = mybir.dt.bfloat16
F32 = mybir.dt.float32
I16 = mybir.dt.int16
AF = mybir.ActivationFunctionType
OP = mybir.AluOpType

NPBF16 = ml_dtypes.bfloat16

THETAS = [[3.0, -3.0, 0.75], [0.0, 3.0, -1.5], [0.0, 0.0, 0.75]]
KPOLY = 3
SUP = 5  # blocks per gather super-group

_src = inspect.getsource(bass.BassGpSimd.dma_gather)
_src = _src.replace("elem_size_bytes > 0 and elem_size_bytes % 256 == 0",
                    "elem_size_bytes > 0")
_ns = dict(vars(bass))
exec("def _dma_gather_relaxed" + _src[_src.index("("):], _ns)
bass.BassGpSimd.dma_gather_relaxed = _ns["_dma_gather_relaxed"]


def _idx_tile(J):
    n = len(J)
    assert n % 16 == 0
    a16 = np.ascontiguousarray(J.reshape(-1, 16).T.astype(np.int16))
    return np.tile(a16, (8, 1))


def host_prep(inputs, n_cores=8, block=128):
    in_feat = np.asarray(inputs["in_feat"], np.float32)
    src = np.asarray(inputs["src"], np.int64)
    dst = np.asarray(inputs["dst"], np.int64)
    N, IN = in_feat.shape
    H = np.asarray(inputs["W1"]).shape[1]

    NS = -(-N // (n_cores * block)) * block
    NP = NS * n_cores
    NB = NS // block
    NBA = (NB + 1) // 2          # blocks in half A (per rank)
    NSA = NBA * block            # rows per rank in table A
    NSB = NS - NSA
    NPA = NSA * n_cores
    NPB = NSB * n_cores
    assert NPA < 32768 and NPB < 32768

    x = np.zeros((NP, IN), np.float32)
    x[:N] = in_feat

    deg = np.bincount(dst, minlength=N).astype(np.float32)
    dinv = np.clip(deg, 1.0, None) ** -0.5
    dinv = np.concatenate([dinv, np.ones(NP - N, np.float32)])

    order = np.argsort(dst, kind="stable")
    ds = dst[order]
    ss = src[order]
    core_of = ds // NS
    loc = ds % NS
    blk = loc // block
    reb = (loc % block).astype(np.float32)
    s_r = ss // NS
    s_l = ss % NS
    half = (s_l >= NSA).astype(np.int64)
    rowA = s_r * NSA + s_l                  # valid when half==0
    rowB = s_r * NSB + (s_l - NSA)          # valid when half==1

    counts = np.zeros((2, n_cores, NB), np.int64)
    for c in range(n_cores):
        m = core_of == c
        for h in (0, 1):
            counts[h, c] = np.bincount(blk[m & (half == h)], minlength=NB)
    nchA = np.maximum(-(-counts[0].max(axis=0) // block), 1).astype(np.int64)
    nchB = (-(-counts[1].max(axis=0) // block)).astype(np.int64)

    supers = [list(range(s, min(s + SUP, NB))) for s in range(0, NB, SUP)]
    slotA = np.zeros(NB, np.int64)
    slotB = np.zeros(NB, np.int64)
    sup0, colA, colB = [], [], []
    g = cA = cB = 0
    for blocks in supers:
        sup0.append(g)
        colA.append(cA)
        colB.append(cB)
        for b in blocks:
            slotA[b] = g
            g += nchA[b]
        for b in blocks:
            slotB[b] = g
            g += nchB[b]
        cA += sum(int(nchA[b]) for b in blocks) * 8
        cB += sum(int(nchB[b]) for b in blocks) * 8
    TCt = int(g)
    TCA8 = max(int(cA), 8)
    TCB8 = max(int(cB), 8)
    maxKA = max(sum(int(nchA[b]) for b in blocks) for blocks in supers)
    maxKB = max(max(sum(int(nchB[b]) for b in blocks) for blocks in supers), 1)

    in_maps = []
    for c in range(n_cores):
        m = core_of == c
        cblk = blk[m]
        crowA = rowA[m]
        crowB = rowB[m]
        creb = reb[m]
        chalf = half[m]
        reb_arr = np.full((128, TCt), 200.0, np.float32)
        iA = np.zeros(TCA8 * 16, np.int64)
        iB = np.zeros(TCB8 * 16, np.int64)
        pA = pB = 0
        for blocks in supers:
            for h, nch_h, slot_h, crow in ((0, nchA, slotA, crowA),
                                           (1, nchB, slotB, crowB)):
                for b in blocks:
                    if nch_h[b] == 0:
                        continue
                    bm = (cblk == b) & (chalf == h)
                    n_e = int(bm.sum())
                    pad = int(nch_h[b]) * block
                    ids = np.zeros(pad, np.int64)
                    ids[:n_e] = crow[bm]
                    r_ids = np.full(pad, 200.0, np.float32)
                    r_ids[:n_e] = creb[bm]
                    s0 = slot_h[b]
                    reb_arr[:, s0:s0 + nch_h[b]] = r_ids.reshape(-1, block).T
                    if h == 0:
                        iA[pA:pA + pad] = ids
                        pA += pad
                    else:
                        iB[pB:pB + pad] = ids
                        pB += pad

        dv = dinv[c * NS:(c + 1) * NS].reshape(NB, block).T.copy()
        Wattn = np.asarray(inputs["Wattn"], np.float32)[:, 0]
        Wf2 = np.asarray(inputs["Wf2"], np.float32)[:, 0]
        im = {
            "xT": np.ascontiguousarray(x[c * NS:(c + 1) * NS].T).astype(NPBF16),
            "dinv": np.ascontiguousarray(dv),
            "ndinv": np.ascontiguousarray(-dv),
            "idxA": _idx_tile(iA),
            "idxB": _idx_tile(iB),
            "rebid": reb_arr.astype(NPBF16),
            "iota": np.tile(np.arange(block, dtype=np.float32), (block, 1)).astype(NPBF16),
            "ident": np.eye(block, dtype=np.float32).astype(NPBF16),
            "W1": np.asarray(inputs["W1"], np.float32).astype(NPBF16),
            "W2": np.asarray(inputs["W2"], np.float32).astype(NPBF16),
            "Wres": np.asarray(inputs["Wres"], np.float32).astype(NPBF16),
            "W3": np.asarray(inputs["W3"], np.float32).astype(NPBF16),
            "W4": np.asarray(inputs["W4"], np.float32).astype(NPBF16),
            "Wf1": np.asarray(inputs["Wf1"], np.float32).astype(NPBF16),
            "WattnR": np.tile(Wattn, (block, 1)).astype(NPBF16),
            "Wf2R": np.tile(Wf2, (block, 1)).astype(NPBF16),
            "bf1R": np.tile(np.asarray(inputs["bf1"], np.float32), (block, 1)),
            "b1": np.asarray(inputs["b1"], np.float32).reshape(-1, 1),
            "b2": np.asarray(inputs["b2"], np.float32).reshape(-1, 1),
            "bres08": 0.8 * np.asarray(inputs["bres"], np.float32).reshape(-1, 1),
            "b3": np.asarray(inputs["b3"], np.float32).reshape(-1, 1),
            "b4": np.asarray(inputs["b4"], np.float32).reshape(-1, 1),
        }
        in_maps.append(im)

    meta = dict(
        N=N, NP=NP, NS=NS, NB=NB, NBA=NBA, NSA=NSA, NSB=NSB, NPA=NPA, NPB=NPB,
        IN=IN, H=H, TCt=TCt, TCA8=TCA8, TCB8=TCB8, maxKA=maxKA, maxKB=maxKB,
        supers=supers,
        nchA=[int(v) for v in nchA], nchB=[int(v) for v in nchB],
        slotA=[int(v) for v in slotA], slotB=[int(v) for v in slotB],
        sup0=sup0, colA=colA, colB=colB,
        battn=float(np.asarray(inputs["battn"]).reshape(-1)[0]),
        bf2=float(np.asarray(inputs["bf2"]).reshape(-1)[0]),
        n_cores=n_cores,
    )
    return in_maps, meta


def build(meta, stage=8):
    NS, NB, IN, H, NP = (meta[k] for k in ("NS", "NB", "IN", "H", "NP"))
    NBA, NSA, NSB, NPA, NPB = (
        meta[k] for k in ("NBA", "NSA", "NSB", "NPA", "NPB"))
    TCt, TCA8, TCB8, maxKA, maxKB = (
        meta[k] for k in ("TCt", "TCA8", "TCB8", "maxKA", "maxKB"))
    supers, nchA, nchB = meta["supers"], meta["nchA"], meta["nchB"]
    slotA, slotB, sup0 = meta["slotA"], meta["slotB"], meta["sup0"]
    colA, colB = meta["colA"], meta["colB"]
    n_cores = meta["n_cores"]
    C = 2
    P = 128
    max_nch_half = max(max(nchA), max(nchB))

    col_tiles = []
    pos = 0
    while pos < NS:
        w = min(512, NS - pos)
        col_tiles.append((pos, w))
        pos += w

    nc = bacc.Bacc("TRN2", num_devices=n_cores, num_swdge_queues=4)

    xT = nc.declare_dram_parameter("xT", [IN, NS], BF16, isOutput=False)
    dinv_d = nc.declare_dram_parameter("dinv", [P, NB], F32, isOutput=False)
    ndinv_d = nc.declare_dram_parameter("ndinv", [P, NB], F32, isOutput=False)
    idxA_d = nc.declare_dram_parameter("idxA", [P, TCA8], I16, isOutput=False)
    idxB_d = nc.declare_dram_parameter("idxB", [P, TCB8], I16, isOutput=False)
    rebid_d = nc.declare_dram_parameter("rebid", [P, TCt], BF16, isOutput=False)
    iota_d = nc.declare_dram_parameter("iota", [P, P], BF16, isOutput=False)
    ident_d = nc.declare_dram_parameter("ident", [P, P], BF16, isOutput=False)
    W1_d = nc.declare_dram_parameter("W1", [IN, H], BF16, isOutput=False)
    W2_d = nc.declare_dram_parameter("W2", [H, H], BF16, isOutput=False)
    Wres_d = nc.declare_dram_parameter("Wres", [H, H], BF16, isOutput=False)
    W3_d = nc.declare_dram_parameter("W3", [H, H], BF16, isOutput=False)
    W4_d = nc.declare_dram_parameter("W4", [H, C], BF16, isOutput=False)
    Wf1_d = nc.declare_dram_parameter("Wf1", [2 * H, H], BF16, isOutput=False)
    WattnR_d = nc.declare_dram_parameter("WattnR", [P, H], BF16, isOutput=False)
    Wf2R_d = nc.declare_dram_parameter("Wf2R", [P, H], BF16, isOutput=False)
    bf1R_d = nc.declare_dram_parameter("bf1R", [P, H], F32, isOutput=False)
    b1_d = nc.declare_dram_parameter("b1", [H, 1], F32, isOutput=False)
    b2_d = nc.declare_dram_parameter("b2", [H, 1], F32, isOutput=False)
    bres08_d = nc.declare_dram_parameter("bres08", [H, 1], F32, isOutput=False)
    b3_d = nc.declare_dram_parameter("b3", [H, 1], F32, isOutput=False)
    b4_d = nc.declare_dram_parameter("b4", [C, 1], F32, isOutput=False)
    out_d = nc.declare_dram_parameter("out", [C, NS], F32, isOutput=True)

    # split y tables (A/B halves), each all-gathered separately
    y0lA = nc.dram_tensor("y0lA", [NSA, P], BF16)
    y0lB = nc.dram_tensor("y0lB", [NSB, P], BF16)
    y0A = nc.dram_tensor("y0A", [NPA, P], BF16, addr_space="Shared")
    y0B = nc.dram_tensor("y0B", [NPB, P], BF16, addr_space="Shared")
    y1lA = nc.dram_tensor("y1lA", [NSA, P], BF16)
    y1lB = nc.dram_tensor("y1lB", [NSB, P], BF16)
    y1A = nc.dram_tensor("y1A", [NPA, P], BF16, addr_space="Shared")
    y1B = nc.dram_tensor("y1B", [NPB, P], BF16, addr_space="Shared")

    rg = [list(range(n_cores))]

    class StopStage(Exception):
        pass

    with tile.TileContext(nc) as tc:
        with (
            tc.tile_pool(name="const", bufs=1) as cp,
            tc.tile_pool(name="big", bufs=1) as bp,
            tc.tile_pool(name="work", bufs=3) as wp,
            tc.tile_pool(name="gat", bufs=2) as gp,
            tc.tile_pool(name="idx", bufs=2) as ip,
            tc.tile_pool(name="psum", bufs=2, space="PSUM") as pp,
            tc.tile_pool(name="psum_s", bufs=2, space="PSUM") as pps,
        ):
          try:
            def cload(dram, shape, dtype, tag):
                t = cp.tile(shape, dtype, tag=tag)
                nc.sync.dma_start(t[:], dram[:])
                return t

            iota_s = cload(iota_d, [P, P], BF16, "iota")
            ident_s = cload(ident_d, [P, P], BF16, "ident")
            W1_s = cload(W1_d, [IN, H], BF16, "W1")
            W2_s = cload(W2_d, [H, H], BF16, "W2")
            Wres_s = cload(Wres_d, [H, H], BF16, "Wres")
            W3_s = cload(W3_d, [H, H], BF16, "W3")
            W4_s = cload(W4_d, [H, C], BF16, "W4")
            Wf1_s = cload(Wf1_d, [2 * H, H], BF16, "Wf1")
            WattnR_s = cload(WattnR_d, [P, H], BF16, "WattnR")
            Wf2R_s = cload(Wf2R_d, [P, H], BF16, "Wf2R")
            bf1R_s = cload(bf1R_d, [P, H], F32, "bf1R")
            b1_s = cload(b1_d, [H, 1], F32, "b1")
            b2_s = cload(b2_d, [H, 1], F32, "b2")
            bres08_s = cload(bres08_d, [H, 1], F32, "bres08")
            b3_s = cload(b3_d, [H, 1], F32, "b3")
            b4_s = cload(b4_d, [C, 1], F32, "b4")
            dinv_s = cload(dinv_d, [P, NB], F32, "dinv")
            ndinv_s = cload(ndinv_d, [P, NB], F32, "ndinv")
            rebid_s = cload(rebid_d, [P, TCt], BF16, "rebid")
            xT_s = cload(xT, [IN, NS], BF16, "xT")

            h1T = bp.tile([H, NS], BF16, tag="h1T")
            hpT = bp.tile([H, NS], BF16, tag="hpT")
            resT = bp.tile([H, NS], BF16, tag="resT")
            h_nm = bp.tile([P, NB, H], BF16, tag="h_nm")
            f1_nm = bp.tile([P, NB, H], BF16, tag="f1_nm")
            f2_nm = bp.tile([P, NB, H], BF16, tag="f2_nm")
            fTall = bp.tile([H, NS], BF16, tag="fTall")
            y_sb = bp.tile([P, NB, P], BF16, tag="y_sb")
            nc.gpsimd.memset(y_sb[:], 0.0)
            q_all = bp.tile([P, KPOLY, NB], F32, tag="q_all")
            sc_all = bp.tile([P, KPOLY, NB], F32, tag="sc_all")
            cco = bp.tile([P, KPOLY, NB], F32, tag="cco")
            zz = bp.tile([P, NB], F32, tag="zz")
            rr = bp.tile([P, NB], F32, tag="rr")
            fw_all = bp.tile([P, NB], F32, tag="fw_all")
            outT = bp.tile([C, NS], F32, tag="outT")

            # ---- phase 1: MLPs (feature-major) ----
            for (pos, w) in col_tiles:
                ps1 = pp.tile([H, 512], F32, tag="pm")
                nc.tensor.matmul(ps1[:, :w], W1_s[:], xT_s[:, pos:pos + w],
                                 start=True, stop=True)
                nc.scalar.activation(h1T[:, pos:pos + w], ps1[:, :w], AF.Relu,
                                     bias=b1_s[:, 0:1])
                ps2 = pp.tile([H, 512], F32, tag="pm")
                nc.tensor.matmul(ps2[:, :w], W2_s[:], h1T[:, pos:pos + w],
                                 start=True, stop=True)
                nc.scalar.activation(hpT[:, pos:pos + w], ps2[:, :w], AF.Relu,
                                     bias=b2_s[:, 0:1])
                ps3 = pp.tile([H, 512], F32, tag="pm")
                nc.tensor.matmul(ps3[:, :w], Wres_s[:], hpT[:, pos:pos + w],
                                 start=True, stop=True)
                nc.scalar.activation(resT[:, pos:pos + w], ps3[:, :w], AF.Identity,
                                     bias=bres08_s[:, 0:1], scale=0.8)

            if stage < 2:
                raise StopStage

            # ---- phase 2: transpose to node-major + y0 (A blocks first) ----
            def emit_y_writes(lA, lB):
                nc.sync.dma_start(
                    lA.ap().rearrange("(b p) f -> p b f", p=P), y_sb[:, :NBA, :])
                nc.sync.dma_start(
                    lB.ap().rearrange("(b p) f -> p b f", p=P), y_sb[:, NBA:, :])

            for b in range(NB):
                pt = pp.tile([P, H], BF16, tag="pt")
                nc.tensor.transpose(pt[:], hpT[:, b * P:(b + 1) * P], ident_s[:H, :H])
                nc.scalar.activation(h_nm[:, b, :], pt[:], AF.Copy)
                nc.vector.tensor_scalar(out=y_sb[:, b, :H], in0=pt[:],
                                        scalar1=dinv_s[:, b:b + 1], scalar2=None,
                                        op0=OP.mult)
            emit_y_writes(y0lA, y0lB)
            nc.gpsimd.collective_compute(
                "AllGather", OP.bypass, replica_groups=rg,
                ins=[y0lA.ap().opt()], outs=[y0A.ap().opt()])
            nc.gpsimd.collective_compute(
                "AllGather", OP.bypass, replica_groups=rg,
                ins=[y0lB.ap().opt()], outs=[y0B.ap().opt()])

            if stage < 3:
                raise StopStage

            def spmm(tA, tB, x_nm, out_nm, post_block=None):
                tabA = tA.ap()[:, :H]
                tabB = tB.ap()[:, :H]
                for si, blocks in enumerate(supers):
                    KA = sum(nchA[b] for b in blocks)
                    KB = sum(nchB[b] for b in blocks)
                    gA = gp.tile([P, maxKA, H], BF16, tag="GA")
                    iA = ip.tile([P, maxKA * 8], I16, tag="IA")
                    nc.sync.dma_start(iA[:, :KA * 8],
                                      idxA_d[:, colA[si]:colA[si] + KA * 8])
                    nc.gpsimd.dma_gather_relaxed(
                        gA[:, :KA, :], tabA, iA[:, :KA * 8],
                        KA * P, KA * P, H, elem_step=P, single_packet=False,
                        queue_num=(2 * si) % 4)
                    gB = None
                    if KB > 0:
                        gB = gp.tile([P, maxKB, H], BF16, tag="GB")
                        iB = ip.tile([P, maxKB * 8], I16, tag="IB")
                        nc.sync.dma_start(iB[:, :KB * 8],
                                          idxB_d[:, colB[si]:colB[si] + KB * 8])
                        nc.gpsimd.dma_gather_relaxed(
                            gB[:, :KB, :], tabB, iB[:, :KB * 8],
                            KB * P, KB * P, H, elem_step=P, single_packet=False,
                            queue_num=(2 * si + 1) % 4)
                    for b in blocks:
                        groups = []
                        if nchA[b] > 0:
                            groups.append((slotA[b], nchA[b], gA,
                                           slotA[b] - sup0[si]))
                        if nchB[b] > 0:
                            groups.append((slotB[b], nchB[b], gB,
                                           slotB[b] - sup0[si] - KA))
                        ps = pps.tile([P, H], F32, tag="ps")
                        nck = sum(g[1] for g in groups)
                        j = 0
                        for (g0, gn, gbuf, l0) in groups:
                            s_t = wp.tile([P, max_nch_half, P], BF16, tag="S")
                            nc.vector.tensor_tensor(
                                out=s_t[:, :gn, :],
                                in0=iota_s[:].rearrange("p (o m) -> p o m", o=1)
                                    .to_broadcast([P, gn, P]),
                                in1=rebid_s[:, g0:g0 + gn]
                                    .rearrange("p (k o) -> p k o", o=1)
                                    .to_broadcast([P, gn, P]),
                                op=OP.is_equal)
                            for k in range(gn):
                                nc.tensor.matmul(ps[:], s_t[:, k, :],
                                                 gbuf[:, l0 + k, :],
                                                 start=(j == 0),
                                                 stop=(j == nck - 1))
                                j += 1
                        nc.vector.scalar_tensor_tensor(
                            out=out_nm[:, b, :], in0=ps[:],
                            scalar=ndinv_s[:, b:b + 1], in1=x_nm[:, b, :],
                            op0=OP.mult, op1=OP.add)
                    if post_block is not None:
                        post_block(blocks)

            # ---- SPMM 1 -> f1; y1 writes per block ----
            def post1(blocks):
                for b in blocks:
                    nc.vector.tensor_scalar(out=y_sb[:, b, :H],
                                            in0=f1_nm[:, b, :],
                                            scalar1=dinv_s[:, b:b + 1],
                                            scalar2=None, op0=OP.mult)
            spmm(y0A, y0B, h_nm, f1_nm, post_block=post1)
            emit_y_writes(y1lA, y1lB)
            nc.gpsimd.collective_compute(
                "AllGather", OP.bypass, replica_groups=rg,
                ins=[y1lA.ap().opt()], outs=[y1A.ap().opt()])
            nc.gpsimd.collective_compute(
                "AllGather", OP.bypass, replica_groups=rg,
                ins=[y1lB.ap().opt()], outs=[y1B.ap().opt()])

            if stage < 4:
                raise StopStage

            fmaps = [h_nm, f1_nm, f2_nm]
            # q_k for k<2 only needs h/f1: overlaps AG2 + spmm2
            for b in range(NB):
                for k in range(2):
                    scr = wp.tile([P, H], BF16, tag="scr")
                    nc.vector.tensor_tensor(out=scr[:], in0=fmaps[k][:, b, :],
                                            in1=WattnR_s[:], op=OP.mult)
                    nc.vector.tensor_reduce(out=q_all[:, k, b:b + 1], in_=scr[:],
                                            axis=mybir.AxisListType.X, op=OP.add)

            if stage < 5:
                raise StopStage

            # ---- SPMM 2 -> f2, with per-super attention+fusion tail ----
            battn = meta["battn"]
            bf2 = meta["bf2"]
            mcoef = [sum(THETAS[i][k] for i in range(KPOLY)) / KPOLY
                     for k in range(KPOLY)]
            assert abs(mcoef[0] - 1.0) < 1e-12 and abs(mcoef[1]) < 1e-12 \
                and abs(mcoef[2]) < 1e-12, "general mean path not implemented"

            def post2(blocks):
                b0, b1 = blocks[0], blocks[-1] + 1
                sl = slice(b0, b1)
                # q2 for these blocks
                for b in blocks:
                    scr = wp.tile([P, H], BF16, tag="scr")
                    nc.vector.tensor_tensor(out=scr[:], in0=f2_nm[:, b, :],
                                            in1=WattnR_s[:], op=OP.mult)
                    nc.vector.tensor_reduce(out=q_all[:, 2, b:b + 1], in_=scr[:],
                                            axis=mybir.AxisListType.X, op=OP.add)
                # batched softmax over this super's slice (contiguous)
                for i in range(KPOLY):
                    nc.vector.tensor_scalar(
                        out=sc_all[:, i, sl], in0=q_all[:, 0, sl],
                        scalar1=THETAS[i][0], scalar2=battn, op0=OP.mult,
                        op1=OP.add)
                    for k in range(1, KPOLY):
                        if THETAS[i][k] != 0.0:
                            nc.vector.scalar_tensor_tensor(
                                out=sc_all[:, i, sl], in0=q_all[:, k, sl],
                                scalar=THETAS[i][k], in1=sc_all[:, i, sl],
                                op0=OP.mult, op1=OP.add)
                    nc.scalar.activation(sc_all[:, i, sl], sc_all[:, i, sl],
                                         AF.Exp)
                nc.vector.tensor_tensor(out=zz[:, sl], in0=sc_all[:, 0, sl],
                                        in1=sc_all[:, 1, sl], op=OP.add)
                nc.vector.tensor_tensor(out=zz[:, sl], in0=zz[:, sl],
                                        in1=sc_all[:, 2, sl], op=OP.add)
                nc.vector.reciprocal(rr[:, sl], zz[:, sl])
                for k in range(KPOLY):
                    first = True
                    for i in range(KPOLY):
                        if THETAS[i][k] == 0.0:
                            continue
                        if first:
                            nc.vector.tensor_scalar(
                                out=cco[:, k, sl], in0=sc_all[:, i, sl],
                                scalar1=THETAS[i][k], scalar2=None, op0=OP.mult)
                            first = False
                        else:
                            nc.vector.scalar_tensor_tensor(
                                out=cco[:, k, sl], in0=sc_all[:, i, sl],
                                scalar=THETAS[i][k], in1=cco[:, k, sl],
                                op0=OP.mult, op1=OP.add)
                    nc.vector.tensor_tensor(out=cco[:, k, sl],
                                            in0=cco[:, k, sl],
                                            in1=rr[:, sl], op=OP.mult)
                # fusion per block
                for b in blocks:
                    fi = wp.tile([P, 2 * H], BF16, tag="fi")
                    nc.vector.tensor_scalar(
                        out=fi[:, :H], in0=h_nm[:, b, :],
                        scalar1=cco[:, 0, b:b + 1], scalar2=None, op0=OP.mult)
                    for k in range(1, KPOLY):
                        nc.vector.scalar_tensor_tensor(
                            out=fi[:, :H], in0=fmaps2[k][:, b, :],
                            scalar=cco[:, k, b:b + 1], in1=fi[:, :H],
                            op0=OP.mult, op1=OP.add)
                    nc.scalar.activation(fi[:, H:], h_nm[:, b, :], AF.Copy)
                    pf = pp.tile([2 * H, P], BF16, tag="pt")
                    nc.tensor.matmul(pf[:], fi[:], ident_s[:], start=True,
                                     stop=True, is_transpose=True)
                    fiT = wp.tile([2 * H, P], BF16, tag="fiT")
                    nc.scalar.activation(fiT[:], pf[:], AF.Copy)
                    ph = pps.tile([P, H], F32, tag="pfh")
                    nc.tensor.matmul(ph[:], fiT[:], Wf1_s[:], start=True,
                                     stop=True)
                    hid = wp.tile([P, H], BF16, tag="hid")
                    nc.vector.tensor_tensor(out=hid[:], in0=ph[:],
                                            in1=bf1R_s[:], op=OP.add)
                    nc.scalar.activation(hid[:], hid[:], AF.Relu)
                    scr2 = wp.tile([P, H], BF16, tag="scr")
                    nc.vector.tensor_tensor(out=scr2[:], in0=hid[:],
                                            in1=Wf2R_s[:], op=OP.mult)
                    nc.vector.tensor_reduce(out=fw_all[:, b:b + 1],
                                            in_=scr2[:],
                                            axis=mybir.AxisListType.X,
                                            op=OP.add)
                    nc.scalar.activation(fw_all[:, b:b + 1],
                                         fw_all[:, b:b + 1],
                                         AF.Sigmoid, bias=bf2)
                    fus = wp.tile([P, H], BF16, tag="fus")
                    nc.vector.scalar_tensor_tensor(
                        out=fus[:], in0=fi[:, :H], scalar=0.1,
                        in1=h_nm[:, b, :], op0=OP.mult, op1=OP.subtract)
                    nc.vector.scalar_tensor_tensor(
                        out=fus[:], in0=fus[:], scalar=fw_all[:, b:b + 1],
                        in1=h_nm[:, b, :], op0=OP.mult, op1=OP.add)
                    pt2 = pp.tile([H, P], BF16, tag="pt")
                    nc.tensor.matmul(pt2[:], fus[:], ident_s[:], start=True,
                                     stop=True, is_transpose=True)
                    nc.vector.tensor_tensor(
                        out=fTall[:, b * P:(b + 1) * P], in0=pt2[:],
                        in1=resT[:, b * P:(b + 1) * P], op=OP.add)

            fmaps2 = fmaps
            if stage < 6:
                def post2(blocks):  # noqa: F811
                    pass
            spmm(y1A, y1B, f1_nm, f2_nm, post_block=post2)

            if stage < 7:
                raise StopStage

            # ---- tail MLP ----
            for (pos, w) in col_tiles:
                ps5 = pp.tile([H, 512], F32, tag="pm")
                nc.tensor.matmul(ps5[:, :w], W3_s[:], fTall[:, pos:pos + w],
                                 start=True, stop=True)
                o3 = wp.tile([H, 512], BF16, tag="o3")
                nc.scalar.activation(o3[:, :w], ps5[:, :w], AF.Relu,
                                     bias=b3_s[:, 0:1])
                ps6 = pp.tile([C, 512], F32, tag="pm")
                nc.tensor.matmul(ps6[:, :w], W4_s[:], o3[:, :w],
                                 start=True, stop=True)
                nc.scalar.activation(outT[:, pos:pos + w], ps6[:, :w], AF.Identity,
                                     bias=b4_s[:, 0:1])

            nc.sync.dma_start(out_d[:], outT[:])
          except StopStage:
            nc.vector.memset(outT[:], 0.0)
            nc.sync.dma_start(out_d[:], outT[:])

    nc.compile()
    return nc




def kernel(**inputs):
    """Full-input ADC-GNN forward on 8 TRN2 NeuronCores; returns [N, 2] f32."""
    from concourse.bass_utils import run_bass_kernel_spmd
    in_maps, meta = host_prep(inputs)
    nc = build(meta)
    res = run_bass_kernel_spmd(nc, in_maps, core_ids=list(range(meta["n_cores"])))
    full = np.concatenate(
        [res.results[c]["out"].T for c in range(meta["n_cores"])], axis=0)
    return np.ascontiguousarray(full[:meta["N"]]).astype(np.float32)

